# revision 39
# baseline (speedup 1.0000x reference)
"""Single-head causal attention with RoPE + padding mask, data-parallel
over batch across 8 TRN2 NeuronCores (one batch element per core).

Per core (T=4096, C=128, HS=64):
  q = rope(x @ Wq); k = rope(x @ Wk); v = x @ Wv
  S^T[j,i] = k[j]·q[i]           (scores, transposed layout: partition=j)
  P^T = exp(S^T/sqrt(C)) * tri(i>=j)   (no max-subtraction: scores are
        O(0.1) for this problem so exp is numerically safe)
  outT[d,i] = sum_j (mask[j]*v[j,d]) P^T[j,i]; rowsum via a mask column
        appended to v (padding mask applied on the v/rowsum side — exactly
        equivalent to masking scores, and keeps the S matmul at K=64)
  out[i,d] = outT[d,i] / rowsum[i]

Performance structure:
  - bf16 TensorE compute, fp32 PSUM accumulate, fp32 exp on ScalarE.
  - S^T matmuls row-packed in concurrent pairs via tile_position (0,0)/
    (64,0) with q/k duplicated into partitions 64-127 (K=64 -> 2x).
  - i-chunks processed in interleaved pairs so ScalarE (the exp engine,
    the steady-state bottleneck) never drains at chunk seams; projection/
    rope blocks and epilogues are woven between score groups.

Host-side prep is pure layout/precision prep:
  - x passed pre-transposed per-core as xT [C,T] bf16.
  - RoPE pair-swap folded into extra weight matrices Wq_swap/Wk_swap
    (swap adjacent columns), so rope = qraw*cos2 + qswap*sin2s with
    cos2/sin2s passed pre-expanded [HS,T] from host.
  - Output returned in [t%128, t//128, d] layout; host reassembles.
"""

import numpy as np

T, C, HS = 4096, 128, 64
N_CORES = 8
NT = T // 128      # 32 j-tiles of 128
NCH = T // 512     # 8 i-chunks of 512
JGRP = 2           # j-tiles per exp group (PSUM-bank budget bound)
SCALE = float(1.0 / np.sqrt(np.float32(C)))
NEG = -1e30

_CACHE = {}


def _install_tile_drain_patch(tile_mod):
    """This container's walrus rejects instructions with >2 sem waits; split
    Tile's final global drain into one drain per ticked processor."""
    import bass_rust
    from concourse.vector_clock import ScopedClock

    def _patched(self, tick_clock, wait_clock):
        gc = tick_clock.global_clock
        for i in range(len(gc)):
            if gc[i] <= 0:
                continue
            v = bass_rust.VectorClock()
            v.require_at_least(i, gc[i])
            d = self.nc.sync.drain()
            wait_clock.add_sem_waits(d.ins, ScopedClock({None: v}))
        self.nc.all_engine_barrier()
        assert self.sems is not None
        popped = self.nc._tile_sem_poison_stack.pop()
        assert popped is self._sem_poison
        self.nc.clear_and_free_semaphores(list(self.sems.allocated().values()))
        self.nc.all_engine_barrier()

    tile_mod.TileContext._drain_and_barrier = _patched


def _split_excess_waits(nc, mybir, limit=1):
    """This container's walrus rejects instructions with >limit sem waits.
    Hoist excess waits onto standalone EventSemaphore instructions inserted
    just before the offending instruction on the same engine queue."""
    ctr = 0
    for f in nc.m.functions:
        for b in f.blocks:
            il = b.instructions
            out = []
            changed = False
            for ins in il:
                si = ins.sync_info
                waits = list(si.on_wait) if si and si.on_wait else []
                if len(waits) > limit:
                    changed = True
                    excess = waits[: len(waits) - limit]
                    keep = waits[len(waits) - limit :]
                    for i in range(0, len(excess), limit):
                        chunk = excess[i : i + limit]
                        ev = mybir.InstEventSemaphore(
                            name=f"I-waitsplit-{ctr}",
                            engine=ins.engine,
                            ins=[],
                            outs=[],
                            sync_info=mybir.SyncInfo(on_wait=chunk, on_update=[]),
                        )
                        ctr += 1
                        nc.register_instruction(ev)
                        out.append(ev)
                    si.on_wait = keep
                out.append(ins)
            if changed:
                b.instructions = out


def _build_nc():
    import concourse.bass as bass
    import concourse.mybir as mybir
    from concourse import tile, masks

    _install_tile_drain_patch(tile)

    DT = mybir.dt
    F32, BF16 = DT.float32, DT.bfloat16
    AF = mybir.ActivationFunctionType
    ALU = mybir.AluOpType

    nc = bass.Bass()
    xT_e = nc.declare_dram_parameter("xT", [C, T], BF16, isOutput=False)
    # w packed: [C, 5, HS] = wq, wq_swap, wk, wk_swap, wv
    w_e = nc.declare_dram_parameter("w", [C, 5 * HS], BF16, isOutput=False)
    cos2_e = nc.declare_dram_parameter("cos2", [HS, T], F32, isOutput=False)
    sin2s_e = nc.declare_dram_parameter("sin2s", [HS, T], F32, isOutput=False)
    mask01_e = nc.declare_dram_parameter("mask01", [128, NT], F32, isOutput=False)
    # out in [t%128, t//128, d] layout (contiguous per partition); host
    # reassembles to [T, HS]
    out_e = nc.declare_dram_parameter("out", [128, NT, HS], F32, isOutput=True)

    with tile.TileContext(nc) as tc:
        with (
            tc.tile_pool(name="const", bufs=1) as cpool,
            tc.tile_pool(name="work", bufs=3) as wpool,
            tc.tile_pool(name="ps", bufs=2, space="PSUM") as ps,
        ):
            # ---- constants / inputs in SBUF: small critical DMAs first so
            # chunk-0 compute starts ASAP, then the big tails ----
            xT = cpool.tile([C, T], BF16)
            w_sb = cpool.tile([C, 5, HS], BF16)
            mask01 = cpool.tile([128, NT], F32)
            cos2 = cpool.tile([HS, T], F32)
            sin2s = cpool.tile([HS, T], F32)
            nc.sync.dma_start(out=w_sb[:, :, :], in_=w_e.rearrange("c (a d) -> c a d", a=5))
            nc.sync.dma_start(out=mask01[:, :], in_=mask01_e[:, :])
            for ch in range(NCH):
                sl = slice(ch * 512, (ch + 1) * 512)
                nc.sync.dma_start(out=xT[:, sl], in_=xT_e[:, sl])
                nc.sync.dma_start(out=cos2[:, sl], in_=cos2_e[:, sl])
                nc.sync.dma_start(out=sin2s[:, sl], in_=sin2s_e[:, sl])

            identity = cpool.tile([128, 128], F32)
            masks.make_identity(nc, identity[:, :])


            # q2/k2: rows 0..63 = rope(q/k)^T, rows 64..127 duplicate for
            # row-packed (tile_position) S matmuls
            q2 = cpool.tile([128, T], BF16)
            k2 = cpool.tile([128, T], BF16)

            # v tiles + mask column (mask-weighted rowsum): [t, j_tile, d(65)]
            # padding mask applied to v rows + rowsum column instead of scores:
            # identical softmax result, keeps the S matmul at K=64.
            vplus = cpool.tile([128, NT, HS + 1], BF16)
            nc.vector.tensor_copy(vplus[:, :, HS], mask01[:, :])

            out_stage = cpool.tile([128, NT, HS], F32)

            # ---- software-pipelined: projection blocks run 2 chunks
            # ahead of the attention i-chunk consuming them (attention for
            # i-chunk ic only needs projection chunks <= ic, by causality) ----

            def rope_block(ch):
                sl = slice(ch * 512, (ch + 1) * 512)
                q_ps = ps.tile([HS, 512], F32, tag="proj", bufs=2, name=f"q_ps{ch}")
                nc.tensor.matmul(q_ps[:, :], w_sb[:, 0, :], xT[:, sl], start=True, stop=True)
                qs_ps = ps.tile([HS, 512], F32, tag="proj", bufs=2, name=f"qs_ps{ch}")
                nc.tensor.matmul(qs_ps[:, :], w_sb[:, 1, :], xT[:, sl], start=True, stop=True)
                m1 = wpool.tile([HS, 512], BF16, tag="rope", bufs=6, name=f"m1_{ch}")
                nc.vector.tensor_mul(m1[:, :], q_ps[:, :], cos2[:, sl])
                m2 = wpool.tile([HS, 512], BF16, tag="rope", bufs=6, name=f"m2_{ch}")
                nc.vector.tensor_mul(m2[:, :], qs_ps[:, :], sin2s[:, sl])
                nc.vector.tensor_add(q2[0:HS, sl], m1[:, :], m2[:, :])

                k_ps = ps.tile([HS, 512], F32, tag="proj", bufs=2, name=f"k_ps{ch}")
                nc.tensor.matmul(k_ps[:, :], w_sb[:, 2, :], xT[:, sl], start=True, stop=True)
                ks_ps = ps.tile([HS, 512], F32, tag="proj", bufs=2, name=f"ks_ps{ch}")
                nc.tensor.matmul(ks_ps[:, :], w_sb[:, 3, :], xT[:, sl], start=True, stop=True)
                m3 = wpool.tile([HS, 512], BF16, tag="rope", bufs=6, name=f"m3_{ch}")
                nc.vector.tensor_mul(m3[:, :], k_ps[:, :], cos2[:, sl])
                m4 = wpool.tile([HS, 512], BF16, tag="rope", bufs=6, name=f"m4_{ch}")
                nc.vector.tensor_mul(m4[:, :], ks_ps[:, :], sin2s[:, sl])
                nc.vector.tensor_add(k2[0:HS, sl], m3[:, :], m4[:, :])

            def dup_block(ch):
                sl = slice(ch * 512, (ch + 1) * 512)
                nc.gpsimd.dma_start(out=q2[64:128, sl], in_=q2[0:64, sl])
                nc.gpsimd.dma_start(out=k2[64:128, sl], in_=k2[0:64, sl])

            def v_block(ch):
                for tt in range(4):
                    jt = ch * 4 + tt
                    v_ps = ps.tile([128, HS], F32, tag="proj", bufs=2, name=f"v_ps{jt}")
                    nc.tensor.matmul(
                        v_ps[:, :],
                        xT[:, jt * 128 : (jt + 1) * 128],
                        w_sb[:, 4, :],
                        start=True,
                        stop=True,
                    )
                    nc.vector.tensor_scalar_mul(
                        vplus[:, jt, 0:HS], v_ps[:, :], mask01[:, jt : jt + 1]
                    )

            rope_block(0)
            dup_block(0)
            rope_block(1)
            dup_block(1)
            rope_block(2)
            rope_block(3)
            v_block(0)
            v_block(1)

            # ---- pair-interleaved attention: i-chunks (2p, 2p+1) run with
            # their score-groups alternating so ACT always has exp work
            # queued; epilogues of the previous pair and the next projection
            # blocks are woven between groups, off ACT's critical path ----

            def attn_group(ic, g0, outT_ps):
                isl = slice(ic * 512, (ic + 1) * 512)
                njt = 4 * (ic + 1)
                jts = list(range(g0, min(g0 + JGRP, njt)))
                sg_ps = ps.tile(
                    [128, JGRP * 512], F32, tag="sgrp", bufs=2,
                    name=f"sg{ic}_{g0}",
                )
                for idx, jt in enumerate(jts):
                    # row-packed pairs: idx 0 on PE rows 0-63, idx 1 on rows
                    # 64-127 (duplicated q/k) — they run concurrently
                    ro = 64 * (idx % 2)
                    nc.tensor.matmul(
                        sg_ps[:, idx * 512 : (idx + 1) * 512],
                        k2[ro : ro + HS, jt * 128 : (jt + 1) * 128],
                        q2[ro : ro + HS, isl],
                        start=True,
                        stop=True,
                        tile_position=(ro, 0),
                    )
                pt = wpool.tile(
                    [128, JGRP * 512], BF16, tag="pt", bufs=3, name=f"pt{ic}_{g0}"
                )
                nw = len(jts) * 512
                nc.scalar.activation(
                    pt[:, 0:nw], sg_ps[:, 0:nw], AF.Exp, bias=0.0, scale=SCALE
                )
                for idx, jt in enumerate(jts):
                    psl = slice(idx * 512, idx * 512 + 512)
                    tt = jt - 4 * ic
                    if tt >= 0:
                        # diagonal-band j-tile: keep i >= j, i.e.
                        # col - p - 128*tt >= 0 (col in i-chunk, p = j%128)
                        nc.gpsimd.affine_select(
                            out=pt[:, psl],
                            in_=pt[:, psl],
                            compare_op=ALU.is_ge,
                            fill=0.0,
                            base=-128 * tt,
                            pattern=[[1, 512]],
                            channel_multiplier=-1,
                        )
                    nc.tensor.matmul(
                        outT_ps[:, :],
                        vplus[:, jt, :],
                        pt[:, psl],
                        start=(jt == 0),
                        stop=(jt == njt - 1),
                    )

            def epilogue_copy(ic, outT_ps):
                outT_sb = wpool.tile(
                    [HS + 1, 512], F32, tag="outTsb", bufs=4, name=f"oT{ic}"
                )
                nc.vector.tensor_copy(outT_sb[:, :], outT_ps[:, :])
                return outT_sb

            def epilogue_tr(ic, outT_sb):
                for tt in range(4):
                    jt = ic * 4 + tt
                    tr_ps = ps.tile(
                        [128, HS + 1], F32, tag="proj", bufs=2, name=f"tr{jt}"
                    )
                    nc.tensor.matmul(
                        tr_ps[:, :],
                        outT_sb[:, tt * 128 : (tt + 1) * 128],
                        identity[0 : HS + 1, 0 : HS + 1],
                        is_transpose=True,
                        start=True,
                        stop=True,
                    )
                    recip = wpool.tile([128, 1], F32, tag="recip", bufs=2)
                    nc.vector.reciprocal(recip[:, :], tr_ps[:, HS : HS + 1])
                    nc.vector.tensor_scalar_mul(
                        out_stage[:, jt, :], tr_ps[:, 0:HS], recip[:, :]
                    )
                nc.sync.dma_start(
                    out=out_e[:, ic * 4 : ic * 4 + 4, :],
                    in_=out_stage[:, ic * 4 : ic * 4 + 4, :],
                )

            pending = []  # deferred epilogue_tr items: (ic, outT_sb)
            for p in range(NCH // 2):
                ic0, ic1 = 2 * p, 2 * p + 1
                ng0, ng1 = 4 * p + 2, 4 * p + 4
                outT0 = ps.tile(
                    [HS + 1, 512], F32, tag="outT", bufs=2, name=f"outT{ic0}"
                )
                outT1 = ps.tile(
                    [HS + 1, 512], F32, tag="outT", bufs=2, name=f"outT{ic1}"
                )
                # alternate groups of the two i-chunks; ic1's extra groups last
                seq = []
                for g in range(ng1):
                    if g < ng0:
                        seq.append((ic0, g, outT0))
                    seq.append((ic1, g, outT1))
                # insertion points for deferred work, spread over the pair
                inserts = {}
                work = list(pending)
                pending = []
                if p == 0:
                    work += [("dup", 2), ("dup", 3), ("v", 2), ("v", 3)]
                else:
                    if 2 * p + 2 < NCH:
                        work.append(("proj", 2 * p + 2))
                    if 2 * p + 3 < NCH:
                        work.append(("proj", 2 * p + 3))
                step = max(1, len(seq) // (len(work) + 1)) if work else len(seq)
                for i, wk in enumerate(work):
                    inserts.setdefault(min((i + 1) * step, len(seq) - 1), []).append(wk)

                for gi, (ic, g, oT) in enumerate(seq):
                    attn_group(ic, g * JGRP, oT)
                    if ic == ic0 and g == ng0 - 1:
                        sb0 = epilogue_copy(ic0, outT0)
                        pending.append(("tr", ic0, sb0))
                    if ic == ic1 and g == ng1 - 1:
                        sb1 = epilogue_copy(ic1, outT1)
                        pending.append(("tr", ic1, sb1))
                    for wk in inserts.get(gi, []):
                        if wk[0] == "proj":
                            rope_block(wk[1])
                            dup_block(wk[1])
                            v_block(wk[1])
                        elif wk[0] == "dup":
                            dup_block(wk[1])
                        elif wk[0] == "v":
                            v_block(wk[1])
                        else:
                            epilogue_tr(wk[1], wk[2])

            for wk in pending:
                epilogue_tr(wk[1], wk[2])

    _split_excess_waits(nc, mybir, limit=1)
    return nc


def _get_nc():
    if "nc" not in _CACHE:
        _CACHE["nc"] = _build_nc()
    return _CACHE["nc"]


def kernel(x_text_emb, Wq, Wk, Wv, freqs_cos, freqs_sin, x_latex_mask):
    import ml_dtypes
    from concourse.bass_utils import run_bass_kernel_spmd

    bf16 = ml_dtypes.bfloat16
    nc = _get_nc()

    swap = np.arange(HS) ^ 1
    cos2 = np.repeat(np.asarray(freqs_cos, np.float32).T, 2, axis=0)
    sin2s = np.repeat(np.asarray(freqs_sin, np.float32).T, 2, axis=0)
    sin2s[0::2] *= -1.0
    cos2 = np.ascontiguousarray(cos2)
    sin2s = np.ascontiguousarray(sin2s)
    Wq = np.asarray(Wq, np.float32)
    Wk = np.asarray(Wk, np.float32)
    Wv = np.asarray(Wv, np.float32)
    w = np.concatenate([Wq, Wq[:, swap], Wk, Wk[:, swap], Wv], axis=1).astype(bf16)
    w = np.ascontiguousarray(w)
    # mask01[b] laid out [j_in_tile(128), j_tile(NT)]
    mask01 = np.asarray(x_latex_mask != 0, np.float32).reshape(N_CORES, NT, 128)

    in_maps = []
    for b in range(N_CORES):
        in_maps.append(
            {
                "xT": np.ascontiguousarray(np.asarray(x_text_emb[b], np.float32).T).astype(bf16),
                "w": w,
                "cos2": cos2,
                "sin2s": sin2s,
                "mask01": np.ascontiguousarray(mask01[b].T),
            }
        )

    res = run_bass_kernel_spmd(nc, in_maps, core_ids=list(range(N_CORES)))
    # out arrives [128, NT, HS] with row t = a*128+p at [p, a, :]
    out = np.stack(
        [
            np.asarray(res.results[b]["out"], np.float32)
            .transpose(1, 0, 2)
            .reshape(T, HS)
            for b in range(N_CORES)
        ],
        axis=0,
    )
    return out


# revision 41
# speedup vs baseline: 1.1460x; 1.1460x over previous
"""Single-head causal attention with RoPE + padding mask, data-parallel
over batch across 8 TRN2 NeuronCores (one batch element per core).

Per core (T=4096, C=128, HS=64):
  q = rope(x @ Wq); k = rope(x @ Wk); v = x @ Wv
  S^T[j,i] = k[j]·q[i]           (scores, transposed layout: partition=j)
  P^T = exp(S^T/sqrt(C)) * tri(i>=j)   (no max-subtraction: scores are
        O(0.1) for this problem so exp is numerically safe)
  outT[d,i] = sum_j (mask[j]*v[j,d]) P^T[j,i]; rowsum via a mask column
        appended to v (padding mask applied on the v/rowsum side — exactly
        equivalent to masking scores, and keeps the S matmul at K=64)
  out[i,d] = outT[d,i] / rowsum[i]

Performance structure:
  - bf16 TensorE compute, fp32 PSUM accumulate, fp32 exp on ScalarE.
  - S^T matmuls row-packed in concurrent pairs via tile_position (0,0)/
    (64,0) with q/k duplicated into partitions 64-127 (K=64 -> 2x).
  - i-chunks processed in interleaved pairs so ScalarE (the exp engine,
    the steady-state bottleneck) never drains at chunk seams; projection/
    rope blocks and epilogues are woven between score groups.

Host-side prep is pure layout/precision prep:
  - x passed pre-transposed per-core as xT [C,T] bf16.
  - RoPE pair-swap folded into extra weight matrices Wq_swap/Wk_swap
    (swap adjacent columns), so rope = qraw*cos2 + qswap*sin2s with
    cos2/sin2s passed pre-expanded [HS,T] from host.
  - Output returned in [t%128, t//128, d] layout; host reassembles.
"""

import numpy as np

T, C, HS = 4096, 128, 64
N_CORES = 8
NT = T // 128      # 32 j-tiles of 128
NCH = T // 512     # 8 i-chunks of 512
JGRP = 2           # j-tiles per exp group (PSUM-bank budget bound)
SCALE = float(1.0 / np.sqrt(np.float32(C)))
NEG = -1e30

_CACHE = {}


def _install_tile_drain_patch(tile_mod):
    """This container's walrus rejects instructions with >2 sem waits; split
    Tile's final global drain into one drain per ticked processor."""
    import bass_rust
    from concourse.vector_clock import ScopedClock

    def _patched(self, tick_clock, wait_clock):
        gc = tick_clock.global_clock
        for i in range(len(gc)):
            if gc[i] <= 0:
                continue
            v = bass_rust.VectorClock()
            v.require_at_least(i, gc[i])
            d = self.nc.sync.drain()
            wait_clock.add_sem_waits(d.ins, ScopedClock({None: v}))
        self.nc.all_engine_barrier()
        assert self.sems is not None
        popped = self.nc._tile_sem_poison_stack.pop()
        assert popped is self._sem_poison
        self.nc.clear_and_free_semaphores(list(self.sems.allocated().values()))
        self.nc.all_engine_barrier()

    tile_mod.TileContext._drain_and_barrier = _patched


def _split_excess_waits(nc, mybir, limit=1):
    """This container's walrus rejects instructions with >limit sem waits.
    Hoist excess waits onto standalone EventSemaphore instructions inserted
    just before the offending instruction on the same engine queue."""
    ctr = 0
    for f in nc.m.functions:
        for b in f.blocks:
            il = b.instructions
            out = []
            changed = False
            for ins in il:
                si = ins.sync_info
                waits = list(si.on_wait) if si and si.on_wait else []
                if len(waits) > limit:
                    changed = True
                    excess = waits[: len(waits) - limit]
                    keep = waits[len(waits) - limit :]
                    for i in range(0, len(excess), limit):
                        chunk = excess[i : i + limit]
                        ev = mybir.InstEventSemaphore(
                            name=f"I-waitsplit-{ctr}",
                            engine=ins.engine,
                            ins=[],
                            outs=[],
                            sync_info=mybir.SyncInfo(on_wait=chunk, on_update=[]),
                        )
                        ctr += 1
                        nc.register_instruction(ev)
                        out.append(ev)
                    si.on_wait = keep
                out.append(ins)
            if changed:
                b.instructions = out


def _build_nc():
    import concourse.bass as bass
    import concourse.mybir as mybir
    from concourse import tile, masks

    _install_tile_drain_patch(tile)

    DT = mybir.dt
    F32, BF16 = DT.float32, DT.bfloat16
    AF = mybir.ActivationFunctionType
    ALU = mybir.AluOpType

    nc = bass.Bass()
    xT_e = nc.declare_dram_parameter("xT", [C, T], BF16, isOutput=False)
    # w packed: [C, 5, HS] = wq, wq_swap, wk, wk_swap, wv
    w_e = nc.declare_dram_parameter("w", [C, 5 * HS], BF16, isOutput=False)
    cos2_e = nc.declare_dram_parameter("cos2", [HS, T], F32, isOutput=False)
    sin2s_e = nc.declare_dram_parameter("sin2s", [HS, T], F32, isOutput=False)
    mask01_e = nc.declare_dram_parameter("mask01", [128, NT], F32, isOutput=False)
    # out in [t%128, t//128, d] layout (contiguous per partition); host
    # reassembles to [T, HS]
    out_e = nc.declare_dram_parameter("out", [128, NT, HS], F32, isOutput=True)

    with tile.TileContext(nc) as tc:
        with (
            tc.tile_pool(name="const", bufs=1) as cpool,
            tc.tile_pool(name="work", bufs=3) as wpool,
            tc.tile_pool(name="ps", bufs=2, space="PSUM") as ps,
        ):
            # ---- constants / inputs in SBUF: small critical DMAs first so
            # chunk-0 compute starts ASAP, then the big tails ----
            xT = cpool.tile([C, T], BF16)
            w_sb = cpool.tile([C, 5, HS], BF16)
            mask01 = cpool.tile([128, NT], F32)
            cos2 = cpool.tile([HS, T], F32)
            sin2s = cpool.tile([HS, T], F32)
            nc.sync.dma_start(out=w_sb[:, :, :], in_=w_e.rearrange("c (a d) -> c a d", a=5))
            nc.sync.dma_start(out=mask01[:, :], in_=mask01_e[:, :])
            for ch in range(NCH):
                sl = slice(ch * 512, (ch + 1) * 512)
                nc.sync.dma_start(out=xT[:, sl], in_=xT_e[:, sl])
                nc.sync.dma_start(out=cos2[:, sl], in_=cos2_e[:, sl])
                nc.sync.dma_start(out=sin2s[:, sl], in_=sin2s_e[:, sl])

            identity = cpool.tile([128, 128], F32)
            masks.make_identity(nc, identity[:, :])


            # q2/k2: rows 0..63 = rope(q/k)^T, rows 64..127 duplicate for
            # row-packed (tile_position) S matmuls
            q2 = cpool.tile([128, T], BF16)
            k2 = cpool.tile([128, T], BF16)

            # v tiles + mask column (mask-weighted rowsum): [t, j_tile, d(65)]
            # padding mask applied to v rows + rowsum column instead of scores:
            # identical softmax result, keeps the S matmul at K=64.
            vplus = cpool.tile([128, NT, HS + 1], BF16)
            nc.vector.tensor_copy(vplus[:, :, HS], mask01[:, :])

            out_stage = cpool.tile([128, NT, HS], F32)

            # ---- software-pipelined: projection blocks run 2 chunks
            # ahead of the attention i-chunk consuming them (attention for
            # i-chunk ic only needs projection chunks <= ic, by causality) ----

            def rope_block(ch):
                sl = slice(ch * 512, (ch + 1) * 512)
                q_ps = ps.tile([HS, 512], F32, tag="proj", bufs=2, name=f"q_ps{ch}")
                nc.tensor.matmul(q_ps[:, :], w_sb[:, 0, :], xT[:, sl], start=True, stop=True)
                qs_ps = ps.tile([HS, 512], F32, tag="proj", bufs=2, name=f"qs_ps{ch}")
                nc.tensor.matmul(qs_ps[:, :], w_sb[:, 1, :], xT[:, sl], start=True, stop=True)
                m1 = wpool.tile([HS, 512], BF16, tag="rope", bufs=6, name=f"m1_{ch}")
                nc.vector.tensor_mul(m1[:, :], q_ps[:, :], cos2[:, sl])
                m2 = wpool.tile([HS, 512], BF16, tag="rope", bufs=6, name=f"m2_{ch}")
                nc.vector.tensor_mul(m2[:, :], qs_ps[:, :], sin2s[:, sl])
                nc.vector.tensor_add(q2[0:HS, sl], m1[:, :], m2[:, :])
                nc.gpsimd.dma_start(out=q2[64:128, sl], in_=q2[0:64, sl])

                k_ps = ps.tile([HS, 512], F32, tag="proj", bufs=2, name=f"k_ps{ch}")
                nc.tensor.matmul(k_ps[:, :], w_sb[:, 2, :], xT[:, sl], start=True, stop=True)
                ks_ps = ps.tile([HS, 512], F32, tag="proj", bufs=2, name=f"ks_ps{ch}")
                nc.tensor.matmul(ks_ps[:, :], w_sb[:, 3, :], xT[:, sl], start=True, stop=True)
                m3 = wpool.tile([HS, 512], BF16, tag="rope", bufs=6, name=f"m3_{ch}")
                nc.vector.tensor_mul(m3[:, :], k_ps[:, :], cos2[:, sl])
                m4 = wpool.tile([HS, 512], BF16, tag="rope", bufs=6, name=f"m4_{ch}")
                nc.vector.tensor_mul(m4[:, :], ks_ps[:, :], sin2s[:, sl])
                nc.vector.tensor_add(k2[0:HS, sl], m3[:, :], m4[:, :])
                nc.gpsimd.dma_start(out=k2[64:128, sl], in_=k2[0:64, sl])

            def v_block(ch):
                for tt in range(4):
                    jt = ch * 4 + tt
                    v_ps = ps.tile([128, HS], F32, tag="proj", bufs=2, name=f"v_ps{jt}")
                    nc.tensor.matmul(
                        v_ps[:, :],
                        xT[:, jt * 128 : (jt + 1) * 128],
                        w_sb[:, 4, :],
                        start=True,
                        stop=True,
                    )
                    nc.vector.tensor_scalar_mul(
                        vplus[:, jt, 0:HS], v_ps[:, :], mask01[:, jt : jt + 1]
                    )

            rope_block(0)
            rope_block(1)
            v_block(0)
            v_block(1)

            # ---- pair-interleaved attention: i-chunks (2p, 2p+1) run with
            # their score-groups alternating so ACT always has exp work
            # queued; epilogues of the previous pair and the next projection
            # blocks are woven between groups, off ACT's critical path ----

            def attn_group(ic, g0, outT_ps):
                isl = slice(ic * 512, (ic + 1) * 512)
                njt = 4 * (ic + 1)
                jts = list(range(g0, min(g0 + JGRP, njt)))
                sg_ps = ps.tile(
                    [128, JGRP * 512], F32, tag="sgrp", bufs=2,
                    name=f"sg{ic}_{g0}",
                )
                for idx, jt in enumerate(jts):
                    # row-packed pairs: idx 0 on PE rows 0-63, idx 1 on rows
                    # 64-127 (duplicated q/k) — they run concurrently
                    ro = 64 * (idx % 2)
                    nc.tensor.matmul(
                        sg_ps[:, idx * 512 : (idx + 1) * 512],
                        k2[ro : ro + HS, jt * 128 : (jt + 1) * 128],
                        q2[ro : ro + HS, isl],
                        start=True,
                        stop=True,
                        tile_position=(ro, 0),
                    )
                pt = wpool.tile(
                    [128, JGRP * 512], BF16, tag="pt", bufs=4, name=f"pt{ic}_{g0}"
                )
                nw = len(jts) * 512
                nc.scalar.activation(
                    pt[:, 0:nw], sg_ps[:, 0:nw], AF.Exp, bias=0.0, scale=SCALE
                )
                for idx, jt in enumerate(jts):
                    psl = slice(idx * 512, idx * 512 + 512)
                    tt = jt - 4 * ic
                    if tt >= 0:
                        # diagonal-band j-tile: keep i >= j, i.e.
                        # col - p - 128*tt >= 0 (col in i-chunk, p = j%128)
                        nc.gpsimd.affine_select(
                            out=pt[:, psl],
                            in_=pt[:, psl],
                            compare_op=ALU.is_ge,
                            fill=0.0,
                            base=-128 * tt,
                            pattern=[[1, 512]],
                            channel_multiplier=-1,
                        )
                    nc.tensor.matmul(
                        outT_ps[:, :],
                        vplus[:, jt, :],
                        pt[:, psl],
                        start=(jt == 0),
                        stop=(jt == njt - 1),
                    )

            def epilogue_copy(ic, outT_ps):
                outT_sb = wpool.tile(
                    [HS + 1, 512], F32, tag="outTsb", bufs=8, name=f"oT{ic}"
                )
                # split copy: transposes for tiles 0-1 gate on the first
                # half only, shortening the epilogue chain at pair seams
                nc.vector.tensor_copy(outT_sb[:, 0:256], outT_ps[:, 0:256])
                nc.vector.tensor_copy(outT_sb[:, 256:512], outT_ps[:, 256:512])
                return outT_sb

            def epilogue_tr(ic, outT_sb):
                for tt in range(4):
                    jt = ic * 4 + tt
                    tr_ps = ps.tile(
                        [128, HS + 1], F32, tag="proj", bufs=2, name=f"tr{jt}"
                    )
                    nc.tensor.matmul(
                        tr_ps[:, :],
                        outT_sb[:, tt * 128 : (tt + 1) * 128],
                        identity[0 : HS + 1, 0 : HS + 1],
                        is_transpose=True,
                        start=True,
                        stop=True,
                    )
                    recip = wpool.tile([128, 1], F32, tag="recip", bufs=8)
                    nc.vector.reciprocal(recip[:, :], tr_ps[:, HS : HS + 1])
                    nc.vector.tensor_scalar_mul(
                        out_stage[:, jt, :], tr_ps[:, 0:HS], recip[:, :]
                    )
                nc.sync.dma_start(
                    out=out_e[:, ic * 4 : ic * 4 + 4, :],
                    in_=out_stage[:, ic * 4 : ic * 4 + 4, :],
                )

            pending = []  # deferred epilogue_tr items: (ic, outT_sb)
            for p in range(NCH // 2):
                ic0, ic1 = 2 * p, 2 * p + 1
                ng0, ng1 = 4 * p + 2, 4 * p + 4
                outT0 = ps.tile(
                    [HS + 1, 512], F32, tag="outT", bufs=2, name=f"outT{ic0}"
                )
                outT1 = ps.tile(
                    [HS + 1, 512], F32, tag="outT", bufs=2, name=f"outT{ic1}"
                )
                # alternate groups of the two i-chunks; ic1's extra groups last
                seq = []
                for g in range(ng1):
                    if g < ng0:
                        seq.append((ic0, g, outT0))
                    seq.append((ic1, g, outT1))
                # insertion points for deferred work, spread over the pair
                inserts = {}
                work = list(pending)
                pending = []
                if 2 * p + 2 < NCH:
                    work.append(("proj", 2 * p + 2))
                if 2 * p + 3 < NCH:
                    work.append(("proj", 2 * p + 3))
                step = max(1, len(seq) // (len(work) + 1)) if work else len(seq)
                for i, wk in enumerate(work):
                    inserts.setdefault(min((i + 1) * step, len(seq) - 1), []).append(wk)

                for gi, (ic, g, oT) in enumerate(seq):
                    attn_group(ic, g * JGRP, oT)
                    if ic == ic0 and g == ng0 - 1:
                        sb0 = epilogue_copy(ic0, outT0)
                        pending.append(("tr", ic0, sb0))
                    if ic == ic1 and g == ng1 - 1:
                        sb1 = epilogue_copy(ic1, outT1)
                        pending.append(("tr", ic1, sb1))
                    for wk in inserts.get(gi, []):
                        if wk[0] == "proj":
                            rope_block(wk[1])
                            v_block(wk[1])
                        else:
                            epilogue_tr(wk[1], wk[2])

            for wk in pending:
                epilogue_tr(wk[1], wk[2])

    _split_excess_waits(nc, mybir, limit=1)
    return nc


def _get_nc():
    if "nc" not in _CACHE:
        _CACHE["nc"] = _build_nc()
    return _CACHE["nc"]


def kernel(x_text_emb, Wq, Wk, Wv, freqs_cos, freqs_sin, x_latex_mask):
    import ml_dtypes
    from concourse.bass_utils import run_bass_kernel_spmd

    bf16 = ml_dtypes.bfloat16
    nc = _get_nc()

    swap = np.arange(HS) ^ 1
    cos2 = np.repeat(np.asarray(freqs_cos, np.float32).T, 2, axis=0)
    sin2s = np.repeat(np.asarray(freqs_sin, np.float32).T, 2, axis=0)
    sin2s[0::2] *= -1.0
    cos2 = np.ascontiguousarray(cos2)
    sin2s = np.ascontiguousarray(sin2s)
    Wq = np.asarray(Wq, np.float32)
    Wk = np.asarray(Wk, np.float32)
    Wv = np.asarray(Wv, np.float32)
    w = np.concatenate([Wq, Wq[:, swap], Wk, Wk[:, swap], Wv], axis=1).astype(bf16)
    w = np.ascontiguousarray(w)
    # mask01[b] laid out [j_in_tile(128), j_tile(NT)]
    mask01 = np.asarray(x_latex_mask != 0, np.float32).reshape(N_CORES, NT, 128)

    in_maps = []
    for b in range(N_CORES):
        in_maps.append(
            {
                "xT": np.ascontiguousarray(np.asarray(x_text_emb[b], np.float32).T).astype(bf16),
                "w": w,
                "cos2": cos2,
                "sin2s": sin2s,
                "mask01": np.ascontiguousarray(mask01[b].T),
            }
        )

    res = run_bass_kernel_spmd(nc, in_maps, core_ids=list(range(N_CORES)))
    # out arrives [128, NT, HS] with row t = a*128+p at [p, a, :]
    out = np.stack(
        [
            np.asarray(res.results[b]["out"], np.float32)
            .transpose(1, 0, 2)
            .reshape(T, HS)
            for b in range(N_CORES)
        ],
        axis=0,
    )
    return out
